# revision 1
# baseline (speedup 1.0000x reference)
"""Trainium2 Bass kernel: attention layer (B=4, S=2048, D=1024), 8 cores.

Sharding: data-parallel over (batch, query-half): core (b, h) computes
output rows for queries [h*1024, (h+1)*1024) of batch b against all 2048
keys. (A pairwise AllGather to split the K projection across pair cores
was measured: the collective fabric is shared across all 4 pairs, so a
4MB/core exchange costs ~200us wall - rejected.)

The V projection is eliminated by reassociating
O = softmax @ (K_in Wv^T) = (E^T K_in) Wv^T: the A^T = key^T E
contraction (256 bf16 matmuls) + O = A Wv^T (128 bf16 matmuls) replace
Vproj(256) + PV(256), feeding straight off the raw bf16 key input.
Per-core PE work: Qproj 128 + KTproj 256 + scores 256 + A 256 + O 128
= 1024 matmuls (~220us at 213-227ns each). Measured: 277.8us wall,
rel err 2.7e-3 (vs 319-323us for the spill-based baseline).

Numerics: scores are f32r with fp32 PSUM (bf16 scores measured 2.5e-2,
fails the 2e-2 gate); E and the A/O path are bf16 (validated 2.2e-3 in
isolation, 4e-3 with f32r score noise).

Softmax runs in two k-half rounds to keep the ACT exp off the critical
path: round A computes m1 = rowmax(k<8) via ONE gpsimd partition
all-reduce (no halving chain, no DRAM bounce), and exp of tiles 0..7
runs interleaved under the round-B score matmuls. The global max
correction c = exp(m1-m) is folded into the A-phase PSUM drains
(A = c*A_A + A_B), so no E tile is ever rewritten. The A/exp emission
is interleaved pass-by-pass (A_A-qh0, exp-qh0, A_A-qh1, exp-qh1,
A_B-qh0, A_B-qh1) so the in-order DVE queue never holds exp-chain work
ahead of the PSUM-slot-recycling drains; the l-row reduction rides
between the last two passes. All PSUM lives in one 8-bank ring pool
(pool-transition boundary instructions measured 12-15us of stall).

SBUF (per-partition budget 208KB): ST score tiles (16 x 4KB) are
progressively reused via bf16 bitcast views - E[t] lands in the first
half of ST[t-1] (freed by the serial exp chain; E[0] gets its own
tile), A^T tiles in the second halves of ST[0..7], and the bf16 Wv
tiles in ST[8..15] (loaded only after the qh1 exp reads). Long-lived
pools sit on the left SBUF stack, transient phase pools on the right
(the two stacks are independent LIFO allocators).
"""

import numpy as np
import ml_dtypes
from contextlib import ExitStack

import bass_rust
import concourse.bass as bass
import concourse.tile as tile
from concourse import bacc, mybir
from concourse.bass import ts
from concourse.bass_utils import run_bass_kernel_spmd

B, S, D = 4, 2048, 1024
N_CORES = 8
SQ = S // 2            # 1024 query rows per core
SK = S // 2            # key column-wave width
P = 128
NE = D // P            # 8 e-tiles
ND = D // P            # 8 d-tiles
NK = S // P            # 16 k-tiles
NQC = SQ // P          # 8 q-chunks
F32R = mybir.dt.float32r
F32 = mybir.dt.float32
BF16 = mybir.dt.bfloat16

_NC_CACHE = {}


def _build():
    if "nc" in _NC_CACHE:
        return _NC_CACHE["nc"]
    nc = bacc.Bacc("TRN2", target_bir_lowering=False, debug=False,
                   num_devices=N_CORES)

    qT = nc.dram_tensor("qT", [D, SQ], F32R, kind="ExternalInput")
    kT = nc.dram_tensor("kT", [D, S], F32R, kind="ExternalInput")
    keyb = nc.dram_tensor("keyb", [S, D], BF16, kind="ExternalInput")
    wqT = nc.dram_tensor("wqT", [D, D], F32R, kind="ExternalInput")
    wkT = nc.dram_tensor("wkT", [D, D], F32R, kind="ExternalInput")
    wvTb = nc.dram_tensor("wvTb", [D, D], BF16, kind="ExternalInput")
    out = nc.dram_tensor("out", [SQ, D], F32, kind="ExternalOutput")

    from concourse.masks import make_identity

    EXP = mybir.ActivationFunctionType.Exp

    with tile.TileContext(nc) as tc:
        with ExitStack() as ctx:
            dram = ctx.enter_context(tc.tile_pool(name="dram", bufs=1,
                                                  space="DRAM"))
            consts = ctx.enter_context(tc.tile_pool(name="consts", bufs=1,
                                                    side="left"))
            maxp = ctx.enter_context(tc.tile_pool(name="maxp", bufs=1,
                                                  side="left"))
            qtp = ctx.enter_context(tc.tile_pool(name="qtp", bufs=NE,
                                                 side="left"))
            ktsb = ctx.enter_context(tc.tile_pool(name="ktsb", bufs=NE,
                                                  side="left"))

            id8 = consts.tile([8, 8], F32)
            make_identity(nc, id8[:])
            ones_c = consts.tile([P, 1], F32)
            nc.gpsimd.memset(ones_c[:], 1.0)

            macc = maxp.tile([P, SQ], F32)
            m1_bc = maxp.tile([P, SQ], F32)
            m_bc = maxp.tile([P, SQ], F32)
            c_bc = maxp.tile([P, SQ], F32)
            lacc = maxp.tile([P, SQ], F32)
            l_row = maxp.tile([1, SQ], F32)
            e0t = maxp.tile([P, SQ], BF16)
            r8 = maxp.tile([8, P], F32)
            recip_t = maxp.tile([P, 8], F32)

            r_dram = dram.tile([1, SQ], F32)

            QTr = [qtp.tile([P, SQ], F32R, tag="qtr", name=f"qtr{i}")
                   for i in range(NE)]
            KTsb = [ktsb.tile([P, S], F32R, tag="ktsb", name=f"ktsb{e}")
                    for e in range(NE)]

            # PSUM pool for projections + scores (own space stack)
            pmm = ExitStack()
            mm1 = pmm.enter_context(tc.tile_pool(name="mm1", bufs=8,
                                                 space="PSUM"))

            qs = [nc.sync, nc.scalar, nc.gpsimd]

            # ================= Phase 1: Qproj ==========================
            with tc.tile_pool(name="wq", bufs=ND, side="right") as wq, \
                 tc.tile_pool(name="qin", bufs=ND, side="right") as qin:
                wqs = [wq.tile([P, D], F32R, tag="wq", name=f"wq{i}")
                       for i in range(ND)]
                qts = [qin.tile([P, SQ], F32R, tag="qin", name=f"qin{i}")
                       for i in range(ND)]
                for d in range(ND):
                    for h_ in range(2):
                        qs[(4 * d + 2 * h_) % 3].dma_start(
                            wqs[d][:, ts(h_, 512)],
                            wqT.ap()[ts(d, P), ts(h_, 512)])
                        qs[(4 * d + 2 * h_ + 1) % 3].dma_start(
                            qts[d][:, ts(h_, 512)],
                            qT.ap()[ts(d, P), ts(h_, 512)])
                for qh in range(SQ // 512):
                    pss = [mm1.tile([P, 512], F32, tag="mmk",
                                    name=f"psq{e}_{qh}") for e in range(NE)]
                    for d in range(ND):
                        for e in range(NE):
                            nc.tensor.matmul(pss[e][:], wqs[d][:, ts(e, P)],
                                             qts[d][:, ts(qh, 512)],
                                             start=(d == 0),
                                             stop=(d == ND - 1))
                    for e in range(NE):
                        nc.vector.tensor_copy(QTr[e][:, ts(qh, 512)], pss[e][:])

            # ================= Phase 2: KTproj (full) ==================
            # kT streams in two column-waves through a 12-buf ring.
            with tc.tile_pool(name="wk", bufs=ND, side="right") as wk, \
                 tc.tile_pool(name="kin", bufs=12, side="right") as kin:
                wks = [wk.tile([P, D], F32R, tag="wk", name=f"wk{i}")
                       for i in range(ND)]
                kws = {}
                for w in range(2):
                    for d in range(ND):
                        kws[(w, d)] = kin.tile([P, SK], F32R, tag="kin",
                                               name=f"kin{w}_{d}")
                for d in range(ND):
                    qs[(3 * d) % 3].dma_start(wks[d][:, 0:512],
                                              wkT.ap()[ts(d, P), 0:512])
                    qs[(3 * d + 1) % 3].dma_start(wks[d][:, 512:1024],
                                                  wkT.ap()[ts(d, P), 512:1024])
                    qs[(3 * d + 2) % 3].dma_start(kws[(0, d)][:, 0:512],
                                                  kT.ap()[ts(d, P), 0:512])
                for d in range(ND):
                    qs[d % 3].dma_start(kws[(0, d)][:, 512:1024],
                                        kT.ap()[ts(d, P), 512:1024])
                for d in range(ND):
                    for h_ in range(2):
                        qs[(2 * d + h_) % 3].dma_start(
                            kws[(1, d)][:, ts(h_, 512)],
                            kT.ap()[ts(d, P), SK + h_ * 512:SK + h_ * 512 + 512])

                for w in range(2):
                    for kc in range(SK // 512):
                        kcg = w * 2 + kc
                        pss = [mm1.tile([P, 512], F32, tag="mmk",
                                        name=f"psk{e}_{kcg}")
                               for e in range(NE)]
                        for d in range(ND):
                            for e in range(NE):
                                nc.tensor.matmul(
                                    pss[e][:], wks[d][:, ts(e, P)],
                                    kws[(w, d)][:, ts(kc, 512)],
                                    start=(d == 0), stop=(d == ND - 1))
                        for e in range(NE):
                            nc.vector.tensor_copy(KTsb[e][:, ts(kcg, 512)],
                                                  pss[e][:])

            # ========== Phase 3: scores + softmax, two k-rounds ========
            # stp holds the 16 fp32 score tiles; their storage is
            # progressively reused via bf16 bitcast views (ST is only
            # ever read by DVE/ACT, so the f32r-rounding BIR check does
            # not apply):
            #   E[t] (t>=1) -> first bf16 half of ST[t-1] (free after
            #                  exp(t-1) consumed it; serial ACT chain)
            #   E[0]        -> dedicated bf16 tile in the same pool
            #   Asb[dc]     -> second half of ST[dc]     (post-scores)
            #   wvs[d<8]    -> second half of ST[8+d], d=7 -> ST[15] 1st
            pst = ExitStack()
            stp = pst.enter_context(tc.tile_pool(name="stp", bufs=16,
                                                 side="right"))
            STs = {}

            def E_full(t):
                if t == 0:
                    return e0t[:]
                return STs[t - 1][:, 0:512].bitcast(BF16)

            def E_sl(t, qh):
                if t == 0:
                    return e0t[:, ts(qh, 512)]
                return STs[t - 1][:, qh * 256:(qh + 1) * 256].bitcast(BF16)

            def A_sl(dc, lo, ln):
                return STs[dc][:, 512 + lo // 2:
                               512 + (lo + ln) // 2].bitcast(BF16)

            def WV_full(d):
                if d == 7:
                    return STs[15][:, 0:512].bitcast(BF16)
                return STs[8 + d][:, 512:1024].bitcast(BF16)

            def WV_sl(d, lo, ln):
                if d == 7:
                    return STs[15][:, lo // 2:(lo + ln) // 2].bitcast(BF16)
                return STs[8 + d][:, 512 + lo // 2:
                                  512 + (lo + ln) // 2].bitcast(BF16)

            pkr = ExitStack()
            keyring = pkr.enter_context(tc.tile_pool(name="keyring", bufs=8,
                                                     side="right"))

            def score_tile(t, drain_act=False):
                st_t = stp.tile([P, SQ], F32, tag="st", name=f"st{t}")
                STs[t] = st_t
                for qh in range(SQ // 512):
                    ps = mm1.tile([P, 512], F32, tag="mmk",
                                  name=f"pss{t}_{qh}")
                    for e in range(NE):
                        nc.tensor.matmul(ps[:], KTsb[e][:, ts(t, P)],
                                         QTr[e][:, ts(qh, 512)],
                                         start=(e == 0), stop=(e == NE - 1))
                    if drain_act:
                        nc.scalar.copy(st_t[:, ts(qh, 512)], ps[:])
                    else:
                        nc.vector.tensor_copy(st_t[:, ts(qh, 512)], ps[:])
                if t % (NK // 2) == 0:
                    nc.vector.tensor_copy(macc[:], st_t[:])
                else:
                    nc.vector.tensor_max(macc[:], macc[:], st_t[:])
                return st_t

                        # ---- round A: k-tiles 0..7 -> m1 ---------------------------
            # Row-max broadcast via one gpsimd partition all-reduce (no
            # halving chain / DRAM bounce).
            stA = [score_tile(t) for t in range(NK // 2)]
            nc.gpsimd.partition_all_reduce(m1_bc[:], macc[:], channels=P,
                                           reduce_op=bass_rust.ReduceOp.max)

            # ---- round B scores, with round-A exp interleaved ---------
            # Round-B score drains go to the ACT engine so the DVE queue
            # (subs/adds/macc) never blocks them; each interleaved DVE op
            # is ready when reached, so there is no head-of-line stall.
            keyts = {}
            for i, t in enumerate(range(NK // 2, NK)):
                score_tile(t, drain_act=True)
                keyt = keyring.tile([P, D], BF16, tag="keyt",
                                    name=f"keyt{i}")
                nc.sync.dma_start(keyt[:], keyb.ap()[ts(i, P), :])
                keyts[i] = keyt
                st_a = stA[i]
                nc.vector.tensor_sub(st_a[:], st_a[:], m1_bc[:])
                nc.scalar.activation(E_full(i), st_a[:], EXP)
                if i == 1:
                    nc.vector.tensor_add(lacc[:], E_full(0), E_full(1))
                elif i > 1:
                    nc.vector.tensor_add(lacc[:], lacc[:], E_full(i))

            # ---- m, correction factor ---------------------------------
            nc.gpsimd.partition_all_reduce(m_bc[:], macc[:], channels=P,
                                           reduce_op=bass_rust.ReduceOp.max)
            nc.vector.tensor_max(m_bc[:], m_bc[:], m1_bc[:])
            # c = exp(m1 - m) (<= 1), full-tile
            nc.vector.tensor_sub(c_bc[:], m1_bc[:], m_bc[:])
            nc.scalar.activation(c_bc[:], c_bc[:], EXP)
            nc.vector.tensor_mul(lacc[:], lacc[:], c_bc[:])

            # ================= Phase 4: A^T = key^T E ==================
            # A^T[d, q] = sum_k key[k, d] * E[k, q]; bf16 in, fp32 psum.
            # k-split: A_A contracts k=0..7 with the UNcorrected round-A
            # E and applies c = exp(m1-m) on the PSUM drain; A_B
            # contracts k=8..15 and drain-adds. Emission is interleaved
            # so the DVE queue never holds exp-chain work ahead of the
            # PSUM-slot-recycling drains: each pass's drains are emitted
            # immediately after its matmuls, with the next qh-half of
            # the round-B sub/exp chain between passes. The l-adds and
            # l-row reduction (only needed for 1/l at the O drains) are
            # deferred to the back.
            for t in range(NK // 2, NK):
                keyt = keyring.tile([P, D], BF16, tag="keyt",
                                    name=f"keyt{t}")
                nc.sync.dma_start(keyt[:], keyb.ap()[ts(t, P), :])
                keyts[t] = keyt

            def a_pass(qh, trange, name, drain):
                pss = [mm1.tile([P, 512], F32, tag="mmk",
                                name=f"ps{name}{dc}_{qh}")
                       for dc in range(ND)]
                tl = list(trange)
                for t in tl:
                    for dc in range(ND):
                        nc.tensor.matmul(pss[dc][:], keyts[t][:, ts(dc, P)],
                                         E_sl(t, qh),
                                         start=(t == tl[0]),
                                         stop=(t == tl[-1]))
                for dc in range(ND):
                    drain(dc, pss[dc])

            def drain_scale(qh):
                def f(dc, ps):
                    nc.vector.tensor_mul(A_sl(dc, qh * 512, 512), ps[:],
                                         c_bc[:, ts(qh, 512)])
                return f

            def drain_add(qh):
                def f(dc, ps):
                    nc.vector.tensor_add(A_sl(dc, qh * 512, 512),
                                         A_sl(dc, qh * 512, 512), ps[:])
                return f

            def expb_half(qh):
                sl = ts(qh, 512)
                for t in range(NK // 2, NK):
                    st_t = STs[t]
                    nc.vector.tensor_sub(st_t[:, sl], st_t[:, sl],
                                         m_bc[:, sl])
                    nc.scalar.activation(E_sl(t, qh), st_t[:, sl], EXP)

            a_pass(0, range(NK // 2), "a", drain_scale(0))
            expb_half(0)
            a_pass(1, range(NK // 2), "a", drain_scale(1))
            expb_half(1)
            # Wv loads alias ST[8..15] second halves: emit only after
            # expb_half(1) has consumed those fp32 score columns
            for d in range(ND):
                nc.scalar.dma_start(WV_full(d), wvTb.ap()[ts(d, P), :])
            a_pass(0, range(NK // 2, NK), "b", drain_add(0))
            # deferred l accumulation for the round-B tiles
            for t in range(NK // 2, NK):
                nc.vector.tensor_add(lacc[:], lacc[:], E_full(t))
            for lh in range(SQ // 512):
                plt = mm1.tile([P, 512], F32, tag="mmk", name=f"pl{lh}")
                nc.tensor.matmul(plt[0:1, :], ones_c[:], lacc[:, ts(lh, 512)],
                                 start=True, stop=True)
                nc.vector.tensor_copy(l_row[0:1, ts(lh, 512)], plt[0:1, :])
            nc.sync.dma_start(r_dram[:], l_row[:])
            nc.sync.dma_start(r8[:],
                              r_dram[0, :].rearrange("(a b) -> a b", a=8))
            a_pass(1, range(NK // 2, NK), "b", drain_add(1))
            pkr.close()

            # ================= Phase 5: l -> 1/l, then O ===============
            with tc.tile_pool(name="outp", bufs=4, side="right") as outp:
                pt8t = mm1.tile([P, 512], F32, tag="mmk", name="pt8")
                nc.tensor.transpose(pt8t[:, 0:8], r8[:], id8[:])
                nc.vector.reciprocal(recip_t[:], pt8t[:, 0:8])

                # ---- O = A^T.T @ WvT, scaled by 1/l on drain ----------
                for qc in range(NQC):
                    for eh in range(D // 512):
                        ps = mm1.tile([P, 512], F32, tag="mmk",
                                      name=f"pso{qc}_{eh}")
                        for dc in range(ND):
                            nc.tensor.matmul(ps[:], A_sl(dc, qc * P, P),
                                             WV_sl(dc, eh * 512, 512),
                                             start=(dc == 0),
                                             stop=(dc == ND - 1))
                        ot = outp.tile([P, 512], F32, tag="ot",
                                       name=f"ot{qc}_{eh}")
                        nc.vector.tensor_scalar_mul(ot[:], ps[:],
                                                    recip_t[:, qc:qc + 1])
                        eng = [nc.sync, nc.gpsimd,
                               nc.scalar][(2 * qc + eh) % 3]
                        eng.dma_start(out.ap()[ts(qc, P), ts(eh, 512)], ot[:])

            pst.close()
            pmm.close()

    nc.compile()
    _NC_CACHE["nc"] = nc
    return nc


def make_in_maps(query, key, Wq, Wk, Wv):
    query = np.asarray(query, dtype=np.float32)
    key = np.asarray(key, dtype=np.float32)
    wqT = np.ascontiguousarray(np.asarray(Wq, dtype=np.float32).T)
    wkT = np.ascontiguousarray(np.asarray(Wk, dtype=np.float32).T)
    wvTb = np.ascontiguousarray(np.asarray(Wv, dtype=np.float32).T).astype(
        ml_dtypes.bfloat16)
    in_maps = []
    for c in range(N_CORES):
        b, h = c // 2, c % 2
        qTn = np.ascontiguousarray(query[b, h * SQ:(h + 1) * SQ, :].T)
        kTn = np.ascontiguousarray(key[b].T)
        keybn = np.ascontiguousarray(key[b]).astype(ml_dtypes.bfloat16)
        in_maps.append({
            "qT": qTn, "kT": kTn, "keyb": keybn,
            "wqT": wqT, "wkT": wkT, "wvTb": wvTb,
        })
    return in_maps


def assemble_out(res):
    outv = np.empty((B, S, D), dtype=np.float32)
    for c in range(N_CORES):
        b, h = c // 2, c % 2
        outv[b, h * SQ:(h + 1) * SQ, :] = res.results[c]["out"]
    return outv


def kernel(query, key, Wq, Wk, Wv):
    nc = _build()
    in_maps = make_in_maps(query, key, Wq, Wk, Wv)
    res = run_bass_kernel_spmd(nc, in_maps, core_ids=list(range(N_CORES)))
    return assemble_out(res)



# revision 2
# speedup vs baseline: 1.3597x; 1.3597x over previous
"""Trainium2 Bass kernel: attention layer (B=4, S=2048, D=1024), 8 cores.

Sharding: data-parallel over (batch, query-half): core (b, h) computes
output rows for queries [h*1024, (h+1)*1024) of batch b against all 2048
keys. (A pairwise AllGather to split the K projection across pair cores
was measured: the collective fabric is shared across all 4 pairs, so a
4MB/core exchange costs ~200us wall - rejected.)

QK fold: scores = (query Wq^T)(Wk key^T) is reassociated as
T = query G with G = Wq^T Wk folded on the host (weight-weight
preprocessing, like the layout transposes), so the K projection
disappears: raw key^T feeds the score matmul straight from DRAM.
The V projection is likewise eliminated by reassociating
O = softmax @ (key Wv^T) = (E^T key) Wv^T. Per-core PE work:
T 128 + scores 256 + A^T 256 + O 128 = 768 matmuls (~175us at
~227ns effective each), down from 1024 in the Qproj/KTproj variant
(measured 277.9us); zero cross-core duplication remains.

Numerics: scores are f32r with fp32 PSUM (bf16 scores measured 2.5e-2,
fails the 2e-2 gate); kT is consumed raw in f32r, so the K-side has no
projection rounding at all. E and the A/O path are bf16.

Softmax runs in two k-half rounds to keep the ACT exp off the critical
path: round A computes m1 = rowmax(k<8) via ONE gpsimd partition
all-reduce (no halving chain, no DRAM bounce), and exp of tiles 0..7
runs interleaved under the round-B score matmuls. The global max
correction c = exp(m1-m) is folded into the A-phase PSUM drains
(A = c*A_A + A_B), so no E tile is ever rewritten. The A/exp emission
is interleaved pass-by-pass (A_A-qh0, exp-qh0, A_A-qh1, exp-qh1,
A_B-qh0, A_B-qh1) so the in-order DVE queue never holds exp-chain work
ahead of the PSUM-slot-recycling drains; the round-B l-accumulation
rides inside the exp interleave (one [P,512] add per tile) so the
l-row reduction never stalls the PE. All PSUM lives in one 8-bank ring
pool (pool-transition boundary instructions measured 12-15us of stall).

SBUF (per-partition budget ~203KB): ST score tiles (16 x 4KB) are
progressively reused via bf16 bitcast views - E[t] lands in the first
half of ST[t-1] (freed by the serial exp chain; E[0] gets its own
tile), A^T tiles in the second halves of ST[0..7], and the bf16 Wv
tiles in ST[8..15] (loaded only after the qh1 exp reads). Long-lived
pools sit on the left SBUF stack (TTr 32KB + raw kT 64KB + stats), the
G tiles (32KB) ride the left-stack top and are freed after the T
phase, before the right-stack stp/keyring peak (80KB).
"""

import numpy as np
import ml_dtypes
from contextlib import ExitStack

import bass_rust
import concourse.bass as bass
import concourse.tile as tile
from concourse import bacc, mybir
from concourse.bass import ts
from concourse.bass_utils import run_bass_kernel_spmd

B, S, D = 4, 2048, 1024
N_CORES = 8
SQ = S // 2            # 1024 query rows per core
P = 128
NE = D // P            # 8 tiles along the hidden dim
ND = D // P
NK = S // P            # 16 k-tiles
NQC = SQ // P          # 8 q-chunks
F32R = mybir.dt.float32r
F32 = mybir.dt.float32
BF16 = mybir.dt.bfloat16

_NC_CACHE = {}


def _build():
    if "nc" in _NC_CACHE:
        return _NC_CACHE["nc"]
    nc = bacc.Bacc("TRN2", target_bir_lowering=False, debug=False,
                   num_devices=N_CORES)

    qT = nc.dram_tensor("qT", [D, SQ], F32R, kind="ExternalInput")
    kT = nc.dram_tensor("kT", [D, S], F32R, kind="ExternalInput")
    keyb = nc.dram_tensor("keyb", [S, D], BF16, kind="ExternalInput")
    g = nc.dram_tensor("g", [D, D], F32R, kind="ExternalInput")
    wvTb = nc.dram_tensor("wvTb", [D, D], BF16, kind="ExternalInput")
    out = nc.dram_tensor("out", [SQ, D], F32, kind="ExternalOutput")

    from concourse.masks import make_identity

    EXP = mybir.ActivationFunctionType.Exp

    with tile.TileContext(nc) as tc:
        with ExitStack() as ctx:
            dram = ctx.enter_context(tc.tile_pool(name="dram", bufs=1,
                                                  space="DRAM"))
            consts = ctx.enter_context(tc.tile_pool(name="consts", bufs=1,
                                                    side="left"))
            maxp = ctx.enter_context(tc.tile_pool(name="maxp", bufs=1,
                                                  side="left"))
            qtp = ctx.enter_context(tc.tile_pool(name="qtp", bufs=NE,
                                                 side="left"))
            ktsb = ctx.enter_context(tc.tile_pool(name="ktsb", bufs=NE,
                                                  side="left"))

            id8 = consts.tile([8, 8], F32)
            make_identity(nc, id8[:])
            ones_c = consts.tile([P, 1], F32)
            nc.gpsimd.memset(ones_c[:], 1.0)

            macc = maxp.tile([P, SQ], F32)
            m1_bc = maxp.tile([P, SQ], F32)
            m_bc = maxp.tile([P, SQ], F32)
            c_bc = maxp.tile([P, SQ], F32)
            lacc = maxp.tile([P, SQ], F32)
            l_row = maxp.tile([1, SQ], F32)
            e0t = maxp.tile([P, SQ], BF16)
            r8 = maxp.tile([8, P], F32)
            recip_t = maxp.tile([P, 8], F32)

            r_dram = dram.tile([1, SQ], F32)

            # TTr[j] holds T^T rows 128j..128j+127 (T = query @ G)
            TTr = [qtp.tile([P, SQ], F32R, tag="ttr", name=f"ttr{j}")
                   for j in range(NE)]
            # KTsb[j] holds raw key^T rows 128j..128j+127 (all 2048 keys)
            KTsb = [ktsb.tile([P, S], F32R, tag="ktsb", name=f"ktsb{j}")
                    for j in range(NE)]

            # PSUM pool: one 8-bank ring for the whole kernel
            pmm = ExitStack()
            mm1 = pmm.enter_context(tc.tile_pool(name="mm1", bufs=8,
                                                 space="PSUM"))

            qs = [nc.sync, nc.scalar, nc.gpsimd]

            # ================= Phase 1: T = query @ G ==================
            # G/qT chunks interleave in i order so the accumulation
            # chains start as soon as the first tiles land; raw kT
            # chunks queue up right behind them.
            with tc.tile_pool(name="gp", bufs=NE, side="left") as gp, \
                 tc.tile_pool(name="qin", bufs=NE, side="right") as qin:
                gsb = [gp.tile([P, D], F32R, tag="gp", name=f"g{i}")
                       for i in range(NE)]
                qts = [qin.tile([P, SQ], F32R, tag="qin", name=f"qin{i}")
                       for i in range(NE)]
                for i in range(ND):
                    for h_ in range(2):
                        qs[(4 * i + 2 * h_) % 3].dma_start(
                            gsb[i][:, ts(h_, 512)],
                            g.ap()[ts(i, P), ts(h_, 512)])
                        qs[(4 * i + 2 * h_ + 1) % 3].dma_start(
                            qts[i][:, ts(h_, 512)],
                            qT.ap()[ts(i, P), ts(h_, 512)])
                # raw key^T streams straight into its resident pool
                for c in range(S // 512):
                    for j in range(NE):
                        qs[(c + j) % 3].dma_start(
                            KTsb[j][:, ts(c, 512)],
                            kT.ap()[ts(j, P), ts(c, 512)])
                for qh in range(SQ // 512):
                    pss = [mm1.tile([P, 512], F32, tag="mmk",
                                    name=f"pst{j}_{qh}") for j in range(NE)]
                    for i in range(ND):
                        for j in range(NE):
                            nc.tensor.matmul(pss[j][:], gsb[i][:, ts(j, P)],
                                             qts[i][:, ts(qh, 512)],
                                             start=(i == 0),
                                             stop=(i == ND - 1))
                    for j in range(NE):
                        nc.vector.tensor_copy(TTr[j][:, ts(qh, 512)],
                                              pss[j][:])

            # ========== Phase 2: scores + softmax, two k-rounds ========
            # stp holds the 16 fp32 score tiles; their storage is
            # progressively reused via bf16 bitcast views (ST is only
            # ever read by DVE/ACT, so the f32r-rounding BIR check does
            # not apply):
            #   E[t] (t>=1) -> first bf16 half of ST[t-1] (free after
            #                  exp(t-1) consumed it; serial ACT chain)
            #   E[0]        -> dedicated bf16 tile in the same pool
            #   Asb[dc]     -> second half of ST[dc]     (post-scores)
            #   wvs[d<8]    -> second half of ST[8+d], d=7 -> ST[15] 1st
            pst = ExitStack()
            stp = pst.enter_context(tc.tile_pool(name="stp", bufs=16,
                                                 side="right"))
            STs = {}

            def E_full(t):
                if t == 0:
                    return e0t[:]
                return STs[t - 1][:, 0:512].bitcast(BF16)

            def E_sl(t, qh):
                if t == 0:
                    return e0t[:, ts(qh, 512)]
                return STs[t - 1][:, qh * 256:(qh + 1) * 256].bitcast(BF16)

            def A_sl(dc, lo, ln):
                return STs[dc][:, 512 + lo // 2:
                               512 + (lo + ln) // 2].bitcast(BF16)

            def WV_full(d):
                if d == 7:
                    return STs[15][:, 0:512].bitcast(BF16)
                return STs[8 + d][:, 512:1024].bitcast(BF16)

            def WV_sl(d, lo, ln):
                if d == 7:
                    return STs[15][:, lo // 2:(lo + ln) // 2].bitcast(BF16)
                return STs[8 + d][:, 512 + lo // 2:
                                  512 + (lo + ln) // 2].bitcast(BF16)

            pkr = ExitStack()
            keyring = pkr.enter_context(tc.tile_pool(name="keyring", bufs=8,
                                                     side="right"))

            def score_tile(t, drain_act=False):
                st_t = stp.tile([P, SQ], F32, tag="st", name=f"st{t}")
                STs[t] = st_t
                for qh in range(SQ // 512):
                    ps = mm1.tile([P, 512], F32, tag="mmk",
                                  name=f"pss{t}_{qh}")
                    for j in range(NE):
                        nc.tensor.matmul(ps[:], KTsb[j][:, ts(t, P)],
                                         TTr[j][:, ts(qh, 512)],
                                         start=(j == 0), stop=(j == NE - 1))
                    if drain_act:
                        nc.scalar.copy(st_t[:, ts(qh, 512)], ps[:])
                    else:
                        nc.vector.tensor_copy(st_t[:, ts(qh, 512)], ps[:])
                if t % (NK // 2) == 0:
                    nc.vector.tensor_copy(macc[:], st_t[:])
                else:
                    nc.vector.tensor_max(macc[:], macc[:], st_t[:])
                return st_t

            # ---- round A: k-tiles 0..7 -> m1 ---------------------------
            # Row-max broadcast via one gpsimd partition all-reduce (no
            # halving chain / DRAM bounce).
            stA = [score_tile(t) for t in range(NK // 2)]
            nc.gpsimd.partition_all_reduce(m1_bc[:], macc[:], channels=P,
                                           reduce_op=bass_rust.ReduceOp.max)

            # ---- round B scores, with round-A exp interleaved ---------
            # Round-B score drains go to the ACT engine so the DVE queue
            # (subs/adds/macc) never blocks them; each interleaved DVE op
            # is ready when reached, so there is no head-of-line stall.
            keyts = {}
            for i, t in enumerate(range(NK // 2, NK)):
                score_tile(t, drain_act=True)
                keyt = keyring.tile([P, D], BF16, tag="keyt",
                                    name=f"keyt{i}")
                nc.sync.dma_start(keyt[:], keyb.ap()[ts(i, P), :])
                keyts[i] = keyt
                st_a = stA[i]
                nc.vector.tensor_sub(st_a[:], st_a[:], m1_bc[:])
                nc.scalar.activation(E_full(i), st_a[:], EXP)
                if i == 1:
                    nc.vector.tensor_add(lacc[:], E_full(0), E_full(1))
                elif i > 1:
                    nc.vector.tensor_add(lacc[:], lacc[:], E_full(i))

            # ---- m, correction factor ---------------------------------
            nc.gpsimd.partition_all_reduce(m_bc[:], macc[:], channels=P,
                                           reduce_op=bass_rust.ReduceOp.max)
            nc.vector.tensor_max(m_bc[:], m_bc[:], m1_bc[:])
            # c = exp(m1 - m) (<= 1), full-tile
            nc.vector.tensor_sub(c_bc[:], m1_bc[:], m_bc[:])
            nc.scalar.activation(c_bc[:], c_bc[:], EXP)
            nc.vector.tensor_mul(lacc[:], lacc[:], c_bc[:])

            # ================= Phase 3: A^T = key^T E ==================
            # A^T[d, q] = sum_k key[k, d] * E[k, q]; bf16 in, fp32 psum.
            # k-split: A_A contracts k=0..7 with the UNcorrected round-A
            # E and applies c = exp(m1-m) on the PSUM drain; A_B
            # contracts k=8..15 and drain-adds. Emission is interleaved
            # so the DVE queue never holds exp-chain work ahead of the
            # PSUM-slot-recycling drains: each pass's drains are emitted
            # immediately after its matmuls, with the next qh-half of
            # the round-B sub/exp chain between passes. The round-B l
            # adds ride inside the exp interleave (per-tile [P,512]
            # adds), so by the time the l-row matmuls are reached the
            # DVE chain has already drained under the A_B matmuls.
            for t in range(NK // 2, NK):
                keyt = keyring.tile([P, D], BF16, tag="keyt",
                                    name=f"keyt{t}")
                nc.sync.dma_start(keyt[:], keyb.ap()[ts(t, P), :])
                keyts[t] = keyt

            def a_pass(qh, trange, name, drain):
                pss = [mm1.tile([P, 512], F32, tag="mmk",
                                name=f"ps{name}{dc}_{qh}")
                       for dc in range(ND)]
                tl = list(trange)
                for t in tl:
                    for dc in range(ND):
                        nc.tensor.matmul(pss[dc][:], keyts[t][:, ts(dc, P)],
                                         E_sl(t, qh),
                                         start=(t == tl[0]),
                                         stop=(t == tl[-1]))
                for dc in range(ND):
                    drain(dc, pss[dc])

            def drain_scale(qh):
                def f(dc, ps):
                    nc.vector.tensor_mul(A_sl(dc, qh * 512, 512), ps[:],
                                         c_bc[:, ts(qh, 512)])
                return f

            def drain_add(qh):
                def f(dc, ps):
                    nc.vector.tensor_add(A_sl(dc, qh * 512, 512),
                                         A_sl(dc, qh * 512, 512), ps[:])
                return f

            def expb_half(qh):
                sl = ts(qh, 512)
                for t in range(NK // 2, NK):
                    st_t = STs[t]
                    nc.vector.tensor_sub(st_t[:, sl], st_t[:, sl],
                                         m_bc[:, sl])
                    nc.scalar.activation(E_sl(t, qh), st_t[:, sl], EXP)
                    nc.vector.tensor_add(lacc[:, sl], lacc[:, sl],
                                         E_sl(t, qh))

            a_pass(0, range(NK // 2), "a", drain_scale(0))
            expb_half(0)
            a_pass(1, range(NK // 2), "a", drain_scale(1))
            expb_half(1)
            # Wv loads alias ST[8..15] second halves: emit only after
            # expb_half(1) has consumed those fp32 score columns
            for d in range(ND):
                nc.scalar.dma_start(WV_full(d), wvTb.ap()[ts(d, P), :])
            a_pass(0, range(NK // 2, NK), "b", drain_add(0))
            for lh in range(SQ // 512):
                plt = mm1.tile([P, 512], F32, tag="mmk", name=f"pl{lh}")
                nc.tensor.matmul(plt[0:1, :], ones_c[:], lacc[:, ts(lh, 512)],
                                 start=True, stop=True)
                nc.vector.tensor_copy(l_row[0:1, ts(lh, 512)], plt[0:1, :])
            nc.sync.dma_start(r_dram[:], l_row[:])
            nc.sync.dma_start(r8[:],
                              r_dram[0, :].rearrange("(a b) -> a b", a=8))
            a_pass(1, range(NK // 2, NK), "b", drain_add(1))
            pkr.close()

            # ================= Phase 4: l -> 1/l, then O ===============
            with tc.tile_pool(name="outp", bufs=4, side="right") as outp:
                pt8t = mm1.tile([P, 512], F32, tag="mmk", name="pt8")
                nc.tensor.transpose(pt8t[:, 0:8], r8[:], id8[:])
                nc.vector.reciprocal(recip_t[:], pt8t[:, 0:8])

                # ---- O = A^T.T @ WvT, scaled by 1/l on drain ----------
                for qc in range(NQC):
                    for eh in range(D // 512):
                        ps = mm1.tile([P, 512], F32, tag="mmk",
                                      name=f"pso{qc}_{eh}")
                        for dc in range(ND):
                            nc.tensor.matmul(ps[:], A_sl(dc, qc * P, P),
                                             WV_sl(dc, eh * 512, 512),
                                             start=(dc == 0),
                                             stop=(dc == ND - 1))
                        ot = outp.tile([P, 512], F32, tag="ot",
                                       name=f"ot{qc}_{eh}")
                        nc.vector.tensor_scalar_mul(ot[:], ps[:],
                                                    recip_t[:, qc:qc + 1])
                        eng = [nc.sync, nc.gpsimd,
                               nc.scalar][(2 * qc + eh) % 3]
                        eng.dma_start(out.ap()[ts(qc, P), ts(eh, 512)], ot[:])

            pst.close()
            pmm.close()

    nc.compile()
    _NC_CACHE["nc"] = nc
    return nc


def make_in_maps(query, key, Wq, Wk, Wv):
    query = np.asarray(query, dtype=np.float32)
    key = np.asarray(key, dtype=np.float32)
    # G = Wq^T @ Wk folds the Q and K projections into one bilinear
    # form: scores = (q Wq^T)(k Wk^T)^T = q (Wq^T Wk) k^T.
    g = np.ascontiguousarray(
        np.asarray(Wq, dtype=np.float64).T @ np.asarray(Wk, dtype=np.float64)
    ).astype(np.float32)
    wvTb = np.ascontiguousarray(np.asarray(Wv, dtype=np.float32).T).astype(
        ml_dtypes.bfloat16)
    in_maps = []
    for c in range(N_CORES):
        b, h = c // 2, c % 2
        qTn = np.ascontiguousarray(query[b, h * SQ:(h + 1) * SQ, :].T)
        kTn = np.ascontiguousarray(key[b].T)
        keybn = np.ascontiguousarray(key[b]).astype(ml_dtypes.bfloat16)
        in_maps.append({
            "qT": qTn, "kT": kTn, "keyb": keybn,
            "g": g, "wvTb": wvTb,
        })
    return in_maps


def assemble_out(res):
    outv = np.empty((B, S, D), dtype=np.float32)
    for c in range(N_CORES):
        b, h = c // 2, c % 2
        outv[b, h * SQ:(h + 1) * SQ, :] = res.results[c]["out"]
    return outv


def kernel(query, key, Wq, Wk, Wv):
    nc = _build()
    in_maps = make_in_maps(query, key, Wq, Wk, Wv)
    res = run_bass_kernel_spmd(nc, in_maps, core_ids=list(range(N_CORES)))
    return assemble_out(res)
